# revision 6
# baseline (speedup 1.0000x reference)
"""Trainium2 Bass kernel for nn_CRFusion (6x mamba-ish resblocks with per-token
state), data-parallel over 8 NeuronCores.

Sharding: 8 shards = (batch b in 0..3) x (sequence half). Each core gets half a
sample's L=2304 token sequence plus a 6-token halo on the interior side; the
depthwise conv1d (window 3) loses one token of validity per resblock, so after
6 blocks the un-haloed 1152 tokens are exact. No cross-core communication.

Layout on-core: channels on partitions, tokens on the free dim.
  x:[64,Lc]  x1/z/delta/dx/y:[128,Lc]  h:[128, 16*Lc] (s-major slices)

Key tricks:
 - softplus(x) = ln(1+exp(x)); rsqrt(m) = exp(-0.5*ln(m))   (one ACT table set)
 - deltaA_s = exp(-(s+1)*delta) = q^(s+1), q = exp(-delta): DVE power chain
   (generic fallback: 16 ACT exps with per-partition scale A[:,s])
 - Bm/Cm row broadcast to 128 partitions via PE matmul with one-hot selector
   lhsT (K=32) from a base-0 [32,Lc] tile.
"""

from contextlib import ExitStack

import numpy as np
import concourse.bass as bass
import concourse.tile as tile
from concourse import mybir
from concourse.bass_utils import run_bass_kernel_spmd
import bass_rust

F = mybir.ActivationFunctionType
AL = mybir.AluOpType
FP32 = mybir.dt.float32

B, C_IN, HWID = 4, 1024, 48
L = HWID * HWID            # 2304
DM, DI, DTR, DS = 64, 128, 32, 16
NBLK = 6
HALO = NBLK                # one token of conv validity lost per block
LH = L // 2                # 1152
LC = LH + HALO             # 1158
N_CORES = 8
CHUNKS = [(0, 512), (512, 1024), (1024, LC)]

# ---------------------------------------------------------------------------
# Walrus in this container accepts only ONE sync-wait per instruction. Patch
# Tile lowering to spread multi-waits across preceding same-engine NOPs, and
# the tail drain across multiple drains.
# ---------------------------------------------------------------------------
_MAXW = 1

def _split_waits_in_ordered(tc, ordered):
    nc = tc.nc
    for bb_name, insts in ordered.items():
        new_list = []
        for inst in insts:
            si = inst.sync_info
            if si is not None and len(si.on_wait) > _MAXW:
                w = list(si.on_wait)
                extra, keep = w[:-_MAXW], w[-_MAXW:]
                for i in range(0, len(extra), _MAXW):
                    nop = nc.engines[inst.engine].nop(hint="wait_split").ins
                    nop.sync_info = bass_rust.SyncInfo(
                        on_wait=extra[i:i + _MAXW], on_update=[])
                    new_list.append(nop)
                inst.sync_info = bass_rust.SyncInfo(
                    on_wait=keep, on_update=list(si.on_update))
            new_list.append(inst)
        ordered[bb_name] = new_list

_orig_lower = tile.TileContext._lower_ordered_insts

def _patched_lower(self, ordered):
    _split_waits_in_ordered(self, ordered)
    return _orig_lower(self, ordered)

def _patched_drain_and_barrier(self, tick_clock, wait_clock):
    from concourse.vector_clock import ScopedClock
    drain_inst = self.nc.sync.drain()
    wait_clock.add_sem_waits(drain_inst.ins,
                             ScopedClock({None: tick_clock.global_clock}))
    si = drain_inst.ins.sync_info
    if si is not None and len(si.on_wait) > _MAXW:
        w = list(si.on_wait)
        drain_inst.ins.sync_info = bass_rust.SyncInfo(
            on_wait=w[:_MAXW], on_update=list(si.on_update))
        for i in range(_MAXW, len(w), _MAXW):
            d2 = self.nc.sync.drain()
            d2.ins.sync_info = bass_rust.SyncInfo(on_wait=w[i:i + _MAXW],
                                                  on_update=[])
    self.nc.all_engine_barrier()
    popped = self.nc._tile_sem_poison_stack.pop()
    assert popped is self._sem_poison
    self.nc.clear_and_free_semaphores(list(self.sems.allocated().values()))
    self.nc.all_engine_barrier()

def _install_patches():
    tile.TileContext._lower_ordered_insts = _patched_lower
    tile.TileContext._drain_and_barrier = _patched_drain_and_barrier

# ---------------------------------------------------------------------------
# Program build
# ---------------------------------------------------------------------------
_PROGRAM_CACHE = {}


def _emit(nc, tc, T, power_chain):
    ap = lambda t: t.ap()

    with ExitStack() as stk:
        cpool = stk.enter_context(tc.tile_pool(name="consts", bufs=1))
        big = stk.enter_context(tc.tile_pool(name="big", bufs=1))
        work = stk.enter_context(tc.tile_pool(name="work", bufs=2))

        # ---- constants to SBUF ----
        def cload(name, shape):
            t = cpool.tile(shape, FP32, tag=name)
            nc.sync.dma_start(t[:], ap(T[name]))
            return t

        ipT = cload("ipT", [DM, 2 * DI])
        xpT = cload("xpT", [DI, 2 * DTR])      # cols 0:32 delta, 32:64 B|C
        dpT = cload("dpT", [DTR, DI])
        opT = cload("opT", [DI, DM])
        c3T = cload("c3T", [DM, C_IN])
        sel = cload("sel", [2 * DS, 2 * DS * DI])
        cw = cload("cw", [DI, 3])
        cb = cload("cb", [DI, 1])
        dpb = cload("dpb", [DI, 1])
        Dv = cload("Dv", [DI, 1])
        rmsw = cload("rmsw", [DM, 1])
        c1b = cload("c1b", [DM, 1])
        c2b = cload("c2b", [DM, 1])
        Asb = None if power_chain else cload("Asb", [DI, DS])

        ones64 = cpool.tile([DM, 1], FP32, tag="ones64")
        nc.vector.memset(ones64[:], 1.0)
        ones1x64 = cpool.tile([1, DM], FP32, tag="ones1x64")
        nc.vector.memset(ones1x64[:], 1.0)
        epsb = cpool.tile([1, 1], FP32, tag="epsb")
        nc.vector.memset(epsb[:], 1e-5)

        # ---- persistent big tiles ----
        h_sb = big.tile([DI, DS * LC], FP32, tag="h")
        rgbp = big.tile([DM, LC], FP32, tag="rgbp")
        dtep = big.tile([DM, LC], FP32, tag="dtep")
        xcur = big.tile([DM, LC], FP32, tag="xcur")
        xn = big.tile([DM, LC], FP32, tag="xn")
        o6 = big.tile([DM, LC], FP32, tag="o6")
        x1p = big.tile([DI, LC], FP32, tag="x1p")
        x1c = big.tile([DI, LC], FP32, tag="x1c")
        zs = big.tile([DI, LC], FP32, tag="zs")
        delta = big.tile([DI, LC], FP32, tag="delta")
        dxt = big.tile([DI, LC], FP32, tag="dxt")
        yacc = big.tile([DI, LC], FP32, tag="yacc")
        scr = big.tile([DI, LC], FP32, tag="scr")
        scr2 = big.tile([DI, LC], FP32, tag="scr2")
        qbuf = big.tile([DI, LC], FP32, tag="qbuf")
        dA_a = big.tile([DI, LC], FP32, tag="dA_a")
        dA_b = big.tile([DI, LC], FP32, tag="dA_b")
        rs_sb = big.tile([1, LC], FP32, tag="rs_sb")

        # ---- front 1x1 convs: proj = W @ feat + b  (own PSUM pool, closed
        # before the block loop so its 3 banks free up) ----
        with tc.tile_pool(name="frontps", bufs=1, space="PSUM") as frontps:
            for name, wname, bias_t, dst in (("rgb", "c1T", c1b, rgbp),
                                             ("dte", "c2T", c2b, dtep)):
                ps_f = frontps.tile([DM, LC], FP32, tag="front")
                for kt in range(C_IN // DI):
                    ktile = work.tile([DI, LC], FP32, tag="fr_in")
                    nc.sync.dma_start(ktile[:],
                                      ap(T[name])[kt * DI:(kt + 1) * DI, :])
                    wtile = work.tile([DI, DM], FP32, tag="fr_w")
                    nc.sync.dma_start(wtile[:],
                                      ap(T[wname])[kt * DI:(kt + 1) * DI, :])
                    for (c0, c1) in CHUNKS:
                        nc.tensor.matmul(ps_f[:, c0:c1], wtile[:],
                                         ktile[:, c0:c1],
                                         start=(kt == 0),
                                         stop=(kt == C_IN // DI - 1))
                nc.scalar.activation(dst[:], ps_f[:], F.Identity,
                                     bias=bias_t[:])

        mmps = stk.enter_context(
            tc.tile_pool(name="mmps", bufs=4, space="PSUM"))
        bcps = stk.enter_context(
            tc.tile_pool(name="bcps", bufs=4, space="PSUM"))

        # ---- resblocks ----
        for blk in range(NBLK):
            xin = rgbp if blk == 0 else xcur
            # RMSNorm: xn = xin * rsqrt(mean(xin^2)+eps) * rmsw
            nc.vector.tensor_tensor(scr[0:DM, :], xin[:], xin[:], AL.mult)
            for (c0, c1) in CHUNKS:
                w_ = c1 - c0
                ps_r = mmps.tile([DI, 512], FP32, tag="mm")
                nc.tensor.matmul(ps_r[0:1, :w_], ones64[:], scr[0:DM, c0:c1],
                                 start=True, stop=True)
                # rs = ln(sum/64 + eps) for now; exp(-.5 * ) after
                nc.scalar.activation(rs_sb[:, c0:c1], ps_r[0:1, :w_], F.Ln,
                                     scale=1.0 / DM, bias=epsb[:])
            nc.scalar.activation(rs_sb[:], rs_sb[:], F.Exp, scale=-0.5)
            for (c0, c1) in CHUNKS:
                w_ = c1 - c0
                ps_rb = mmps.tile([DI, 512], FP32, tag="mm")
                nc.tensor.matmul(ps_rb[0:DM, :w_], ones1x64[:],
                                 rs_sb[:, c0:c1], start=True, stop=True)
                nc.vector.tensor_tensor(xn[:, c0:c1], xin[:, c0:c1],
                                        ps_rb[0:DM, :w_], AL.mult)
            nc.vector.tensor_scalar_mul(xn[:], xn[:], rmsw[:])

            # in_proj: x1pre / z
            for (c0, c1) in CHUNKS:
                w_ = c1 - c0
                ps_x1 = mmps.tile([DI, 512], FP32, tag="mm")
                ps_z = mmps.tile([DI, 512], FP32, tag="mm")
                nc.tensor.matmul(ps_x1[:, :w_], ipT[:, 0:DI], xn[:, c0:c1],
                                 start=True, stop=True)
                nc.tensor.matmul(ps_z[:, :w_], ipT[:, DI:2 * DI], xn[:, c0:c1],
                                 start=True, stop=True)
                nc.vector.tensor_copy(x1p[:, c0:c1], ps_x1[:, :w_])
                nc.scalar.activation(zs[:, c0:c1], ps_z[:, :w_], F.Silu)

            # depthwise conv1d (window 3, zero pad) + bias, then silu
            nc.vector.tensor_scalar(scr[:], x1p[:], cw[:, 1:2], cb[:],
                                    AL.mult, AL.add)
            nc.vector.scalar_tensor_tensor(scr[:, 1:], x1p[:, :LC - 1],
                                           cw[:, 0:1], scr[:, 1:],
                                           AL.mult, AL.add)
            nc.vector.scalar_tensor_tensor(scr[:, :LC - 1], x1p[:, 1:],
                                           cw[:, 2:3], scr[:, :LC - 1],
                                           AL.mult, AL.add)
            nc.scalar.activation(x1c[:], scr[:], F.Silu)

            # x_proj -> delta_r (scr2 rows 0:32) and BC rows (scr rows 0:32)
            dr_sb = scr2
            bc_sb = scr
            for (c0, c1) in CHUNKS:
                w_ = c1 - c0
                ps_d = mmps.tile([DI, 512], FP32, tag="mm")
                ps_bc = mmps.tile([DI, 512], FP32, tag="mm")
                nc.tensor.matmul(ps_d[0:DTR, :w_], xpT[:, 0:DTR],
                                 x1c[:, c0:c1], start=True, stop=True)
                nc.tensor.matmul(ps_bc[0:2 * DS, :w_], xpT[:, DTR:2 * DTR],
                                 x1c[:, c0:c1], start=True, stop=True)
                nc.scalar.activation(dr_sb[0:DTR, c0:c1], ps_d[0:DTR, :w_],
                                     F.Copy)
                nc.vector.tensor_copy(bc_sb[0:2 * DS, c0:c1],
                                      ps_bc[0:2 * DS, :w_])

            # dt_proj + softplus: delta = ln(1 + exp(xp + dpb)); q = exp(-d)
            for (c0, c1) in CHUNKS:
                w_ = c1 - c0
                ps_dp = mmps.tile([DI, 512], FP32, tag="mm")
                nc.tensor.matmul(ps_dp[:, :w_], dpT[:], dr_sb[0:DTR, c0:c1],
                                 start=True, stop=True)
                nc.scalar.activation(delta[:, c0:c1], ps_dp[:, :w_], F.Exp,
                                     bias=dpb[:])
            nc.vector.tensor_scalar_add(delta[:], delta[:], 1.0)
            nc.scalar.activation(delta[:], delta[:], F.Ln)
            if power_chain and blk > 0:
                nc.scalar.activation(qbuf[:], delta[:], F.Exp, scale=-1.0)

            # dx = delta * x1c
            nc.vector.tensor_tensor(dxt[:], delta[:], x1c[:], AL.mult)

            # state update + y accumulation, per s
            prev_dA = None
            for s in range(DS):
                if blk > 0:
                    if power_chain:
                        if s == 0:
                            dA_s = qbuf
                        else:
                            dA_s = dA_a if (s % 2) else dA_b
                            nc.vector.tensor_tensor(dA_s[:], prev_dA[:],
                                                    qbuf[:], AL.mult)
                    else:
                        dA_s = dA_a if (s % 2) else dA_b
                        nc.scalar.activation(dA_s[:], delta[:], F.Exp,
                                             scale=Asb[:, s:s + 1])
                    prev_dA = dA_s
                for (c0, c1) in CHUNKS:
                    w_ = c1 - c0
                    ps_bmb = bcps.tile([DI, 512], FP32, tag="bc")
                    ps_cmb = bcps.tile([DI, 512], FP32, tag="bc")
                    nc.tensor.matmul(ps_bmb[:, :w_],
                                     sel[:, s * DI:(s + 1) * DI],
                                     bc_sb[0:2 * DS, c0:c1],
                                     start=True, stop=True)
                    nc.tensor.matmul(ps_cmb[:, :w_],
                                     sel[:, (DS + s) * DI:(DS + s + 1) * DI],
                                     bc_sb[0:2 * DS, c0:c1],
                                     start=True, stop=True)
                    hs = h_sb[:, s * LC + c0: s * LC + c1]
                    if blk == 0:
                        nc.vector.tensor_tensor(hs, dxt[:, c0:c1],
                                                ps_bmb[:, :w_], AL.mult)
                    else:
                        nc.vector.tensor_tensor(hs, hs, dA_s[:, c0:c1],
                                                AL.mult)
                        nc.vector.tensor_tensor(scr2[0:DI, c0:c1],
                                                dxt[:, c0:c1], ps_bmb[:, :w_],
                                                AL.mult)
                        nc.vector.tensor_tensor(hs, hs, scr2[0:DI, c0:c1],
                                                AL.add)
                    if s == 0:
                        nc.vector.tensor_tensor(yacc[:, c0:c1], hs,
                                                ps_cmb[:, :w_], AL.mult)
                    else:
                        nc.vector.tensor_tensor(scr2[0:DI, c0:c1], hs,
                                                ps_cmb[:, :w_], AL.mult)
                        nc.vector.tensor_tensor(yacc[:, c0:c1], yacc[:, c0:c1],
                                                scr2[0:DI, c0:c1], AL.add)
                if blk == NBLK - 1:
                    nc.sync.dma_start(ap(T["hout"])[:, s * LC:(s + 1) * LC],
                                      h_sb[:, s * LC:(s + 1) * LC])

            # y += D * x1c ; gate ; out_proj ; residual (+ next source)
            nc.vector.scalar_tensor_tensor(yacc[:], x1c[:], Dv[:], yacc[:],
                                           AL.mult, AL.add)
            nc.vector.tensor_tensor(yacc[:], yacc[:], zs[:], AL.mult)
            dst = o6 if blk == NBLK - 1 else xcur
            src_next = None if blk == NBLK - 1 else (dtep if blk % 2 == 0
                                                     else rgbp)
            for (c0, c1) in CHUNKS:
                w_ = c1 - c0
                ps_o = mmps.tile([DI, 512], FP32, tag="mm")
                nc.tensor.matmul(ps_o[0:DM, :w_], opT[:], yacc[:, c0:c1],
                                 start=True, stop=True)
                nc.vector.tensor_tensor(dst[:, c0:c1], ps_o[0:DM, :w_],
                                        xn[:, c0:c1], AL.add)
                if src_next is not None:
                    nc.vector.tensor_tensor(dst[:, c0:c1], dst[:, c0:c1],
                                            src_next[:, c0:c1], AL.add)

        # ---- final 1x1 conv (bias added on host) ----
        for mt in range(C_IN // DI):
            csb = work.tile([DI, LC], FP32, tag="c3out")
            for (c0, c1) in CHUNKS:
                w_ = c1 - c0
                ps_c = mmps.tile([DI, 512], FP32, tag="mm")
                nc.tensor.matmul(ps_c[:, :w_], c3T[:, mt * DI:(mt + 1) * DI],
                                 o6[:, c0:c1], start=True, stop=True)
                nc.vector.tensor_copy(csb[:, c0:c1], ps_c[:, :w_])
            nc.sync.dma_start(ap(T["outc"])[mt * DI:(mt + 1) * DI, :], csb[:])


def _build_program(power_chain):
    _install_patches()
    nc = bass.Bass("TRN2", target_bir_lowering=False, debug=False)
    T = {}
    def din(name, shape):
        T[name] = nc.dram_tensor(name, shape, FP32, kind="ExternalInput")
    def dout(name, shape):
        T[name] = nc.dram_tensor(name, shape, FP32, kind="ExternalOutput")

    din("rgb", [C_IN, LC])
    din("dte", [C_IN, LC])
    din("c1T", [C_IN, DM])
    din("c2T", [C_IN, DM])
    din("c3T", [DM, C_IN])
    din("c1b", [DM, 1])
    din("c2b", [DM, 1])
    din("rmsw", [DM, 1])
    din("ipT", [DM, 2 * DI])
    din("cw", [DI, 3])
    din("cb", [DI, 1])
    din("xpT", [DI, 2 * DTR])
    din("dpT", [DTR, DI])
    din("dpb", [DI, 1])
    din("opT", [DI, DM])
    din("Dv", [DI, 1])
    din("sel", [2 * DS, 2 * DS * DI])
    if not power_chain:
        din("Asb", [DI, DS])
    dout("outc", [C_IN, LC])
    dout("hout", [DI, DS * LC])

    with tile.TileContext(nc) as tc:
        _emit(nc, tc, T, power_chain)
    return nc


def _get_program(power_chain):
    key = bool(power_chain)
    if key not in _PROGRAM_CACHE:
        _PROGRAM_CACHE[key] = _build_program(power_chain)
    return _PROGRAM_CACHE[key]


# ---------------------------------------------------------------------------
# Host wrapper
# ---------------------------------------------------------------------------

def _prep_in_maps(inputs):
    f32 = lambda a: np.ascontiguousarray(np.asarray(a, dtype=np.float32))
    rgb = f32(inputs["rgb_feat"]).reshape(B, C_IN, L)
    dte = f32(inputs["dte_feat"]).reshape(B, C_IN, L)

    A = -np.exp(np.asarray(inputs["A_log"], np.float64))  # [DI, DS]
    expect = -np.arange(1, DS + 1, dtype=np.float64)[None, :]
    power_chain = bool(np.allclose(A, np.broadcast_to(expect, A.shape),
                                   rtol=1e-5, atol=1e-5))

    # selector lhsT for B/C row broadcasts: one-hot [32, 128] per index
    sel = np.zeros((2 * DS, 2 * DS, DI), np.float32)
    for i in range(2 * DS):
        sel[i, i, :] = 1.0
    sel = sel.reshape(2 * DS, 2 * DS * DI)

    common = {
        "c1T": f32(inputs["conv1_w"]).T,
        "c2T": f32(inputs["conv2_w"]).T,
        "c3T": f32(inputs["conv3_w"]).T,
        "c1b": f32(inputs["conv1_b"]).reshape(DM, 1),
        "c2b": f32(inputs["conv2_b"]).reshape(DM, 1),
        "rmsw": f32(inputs["rms_w"]).reshape(DM, 1),
        "ipT": f32(inputs["in_proj_w"]).T,
        "cw": f32(inputs["conv1d_w"]).reshape(DI, 3),
        "cb": f32(inputs["conv1d_b"]).reshape(DI, 1),
        "xpT": f32(inputs["x_proj_w"]).T,
        "dpT": f32(inputs["dt_proj_w"]).T,
        "dpb": f32(inputs["dt_proj_b"]).reshape(DI, 1),
        "opT": f32(inputs["out_proj_w"]).T,
        "Dv": f32(inputs["D"]).reshape(DI, 1),
        "sel": sel,
    }
    if not power_chain:
        common["Asb"] = A.astype(np.float32)
    common = {k: np.ascontiguousarray(v) for k, v in common.items()}

    in_maps = []
    for core in range(N_CORES):
        b, hh = divmod(core, 2)
        lo = hh * LH - (HALO if hh else 0)
        m = dict(common)
        m["rgb"] = np.ascontiguousarray(rgb[b, :, lo:lo + LC])
        m["dte"] = np.ascontiguousarray(dte[b, :, lo:lo + LC])
        in_maps.append(m)
    return in_maps, power_chain


def _assemble(results, conv3_b):
    out = np.empty((B, C_IN, L), np.float32)
    h = np.empty((B, L, DI, DS), np.float32)
    for core in range(N_CORES):
        b, hh = divmod(core, 2)
        off = HALO if hh else 0
        oc = results[core]["outc"]                     # [C_IN, LC]
        out[b, :, hh * LH:(hh + 1) * LH] = oc[:, off:off + LH]
        hv = results[core]["hout"].reshape(DI, DS, LC)[:, :, off:off + LH]
        h[b, hh * LH:(hh + 1) * LH] = np.transpose(hv, (2, 0, 1))
    out += np.asarray(conv3_b, np.float32)[None, :, None]
    return out.reshape(B, C_IN, HWID, HWID), h


def kernel(**inputs):
    in_maps, power_chain = _prep_in_maps(inputs)
    nc = _get_program(power_chain)
    res = run_bass_kernel_spmd(nc, in_maps, core_ids=list(range(N_CORES)))
    return _assemble(res.results, inputs["conv3_b"])


# revision 8
# speedup vs baseline: 4803.3256x; 4803.3256x over previous
"""Trainium2 Bass kernel for nn_CRFusion (6x mamba-ish resblocks with per-token
state), data-parallel over 8 NeuronCores.

Sharding: 8 shards = (batch b in 0..3) x (sequence half). Each core gets half a
sample's L=2304 token sequence plus a 6-token halo on the interior side; the
depthwise conv1d (window 3) loses one token of validity per resblock, so after
6 blocks the un-haloed 1152 tokens are exact. No cross-core communication.

Layout on-core: channels on partitions, tokens on the free dim.
  x:[64,Lc]  x1/z/delta/dx/y:[128,Lc]  h:[128, 16*Lc] (s-major slices)

Key tricks:
 - softplus(x) = ln(1+exp(x)); rsqrt(m) = exp(-0.5*ln(m))   (one ACT table set)
 - deltaA_s = exp(-(s+1)*delta) = q^(s+1), q = exp(-delta): DVE power chain
   (generic fallback: 16 ACT exps with per-partition scale A[:,s])
 - Bm/Cm row broadcast to 128 partitions via PE matmul with one-hot selector
   lhsT (K=32) from a base-0 [32,Lc] tile.
"""

from contextlib import ExitStack

import numpy as np
import concourse.bass as bass
import concourse.tile as tile
from concourse import mybir
from concourse.bass_utils import run_bass_kernel_spmd
import bass_rust

F = mybir.ActivationFunctionType
AL = mybir.AluOpType
FP32 = mybir.dt.float32

B, C_IN, HWID = 4, 1024, 48
L = HWID * HWID            # 2304
DM, DI, DTR, DS = 64, 128, 32, 16
NBLK = 6
HALO = NBLK                # one token of conv validity lost per block
LH = L // 2                # 1152
LC = LH + HALO             # 1158
N_CORES = 8
CHUNKS = [(0, 512), (512, 1024), (1024, LC)]

# ---------------------------------------------------------------------------
# Walrus in this container accepts only ONE sync-wait per instruction. Patch
# Tile lowering to spread multi-waits across preceding same-engine NOPs, and
# the tail drain across multiple drains.
# ---------------------------------------------------------------------------
_MAXW = 1

def _split_waits_in_ordered(tc, ordered):
    nc = tc.nc
    for bb_name, insts in ordered.items():
        new_list = []
        for inst in insts:
            si = inst.sync_info
            if si is not None and len(si.on_wait) > _MAXW:
                w = list(si.on_wait)
                extra, keep = w[:-_MAXW], w[-_MAXW:]
                for i in range(0, len(extra), _MAXW):
                    nop = nc.engines[inst.engine].nop(hint="wait_split").ins
                    nop.sync_info = bass_rust.SyncInfo(
                        on_wait=extra[i:i + _MAXW], on_update=[])
                    new_list.append(nop)
                inst.sync_info = bass_rust.SyncInfo(
                    on_wait=keep, on_update=list(si.on_update))
            new_list.append(inst)
        ordered[bb_name] = new_list

_orig_lower = tile.TileContext._lower_ordered_insts

def _patched_lower(self, ordered):
    _split_waits_in_ordered(self, ordered)
    return _orig_lower(self, ordered)

def _patched_drain_and_barrier(self, tick_clock, wait_clock):
    from concourse.vector_clock import ScopedClock
    drain_inst = self.nc.sync.drain()
    wait_clock.add_sem_waits(drain_inst.ins,
                             ScopedClock({None: tick_clock.global_clock}))
    si = drain_inst.ins.sync_info
    if si is not None and len(si.on_wait) > _MAXW:
        w = list(si.on_wait)
        drain_inst.ins.sync_info = bass_rust.SyncInfo(
            on_wait=w[:_MAXW], on_update=list(si.on_update))
        for i in range(_MAXW, len(w), _MAXW):
            d2 = self.nc.sync.drain()
            d2.ins.sync_info = bass_rust.SyncInfo(on_wait=w[i:i + _MAXW],
                                                  on_update=[])
    self.nc.all_engine_barrier()
    popped = self.nc._tile_sem_poison_stack.pop()
    assert popped is self._sem_poison
    self.nc.clear_and_free_semaphores(list(self.sems.allocated().values()))
    self.nc.all_engine_barrier()

def _install_patches():
    tile.TileContext._lower_ordered_insts = _patched_lower
    tile.TileContext._drain_and_barrier = _patched_drain_and_barrier

# ---------------------------------------------------------------------------
# Program build
# ---------------------------------------------------------------------------
_PROGRAM_CACHE = {}


def _emit(nc, tc, T, power_chain):
    ap = lambda t: t.ap()

    with ExitStack() as stk:
        cpool = stk.enter_context(tc.tile_pool(name="consts", bufs=1))
        big = stk.enter_context(tc.tile_pool(name="big", bufs=1))
        work = stk.enter_context(tc.tile_pool(name="work", bufs=2))

        # ---- constants to SBUF ----
        def cload(name, shape):
            t = cpool.tile(shape, FP32, tag=name)
            nc.sync.dma_start(t[:], ap(T[name]))
            return t

        ipT = cload("ipT", [DM, 2 * DI])
        xpT = cload("xpT", [DI, 2 * DTR])      # cols 0:32 delta, 32:64 B|C
        dpT = cload("dpT", [DTR, DI])
        opT = cload("opT", [DI, DM])
        c3T = cload("c3T", [DM, C_IN])
        sel = cload("sel", [2 * DS, 2 * DS * DI])
        cw = cload("cw", [DI, 3])
        cb = cload("cb", [DI, 1])
        dpb = cload("dpb", [DI, 1])
        Dv = cload("Dv", [DI, 1])
        rmsw = cload("rmsw", [DM, 1])
        c1b = cload("c1b", [DM, 1])
        c2b = cload("c2b", [DM, 1])
        Asb = None if power_chain else cload("Asb", [DI, DS])

        ones64 = cpool.tile([DM, 1], FP32, tag="ones64")
        nc.vector.memset(ones64[:], 1.0)
        ones1x64 = cpool.tile([1, DM], FP32, tag="ones1x64")
        nc.vector.memset(ones1x64[:], 1.0)
        epsb = cpool.tile([1, 1], FP32, tag="epsb")
        nc.vector.memset(epsb[:], 1e-5)

        # ---- persistent big tiles ----
        h_sb = big.tile([DI, DS * LC], FP32, tag="h")
        rgbp = big.tile([DM, LC], FP32, tag="rgbp")
        dtep = big.tile([DM, LC], FP32, tag="dtep")
        xcur = big.tile([DM, LC], FP32, tag="xcur")
        xn = big.tile([DM, LC], FP32, tag="xn")
        o6 = big.tile([DM, LC], FP32, tag="o6")
        x1p = big.tile([DI, LC], FP32, tag="x1p")
        x1c = big.tile([DI, LC], FP32, tag="x1c")
        zs = big.tile([DI, LC], FP32, tag="zs")
        delta = big.tile([DI, LC], FP32, tag="delta")
        dxt = big.tile([DI, LC], FP32, tag="dxt")
        yacc = big.tile([DI, LC], FP32, tag="yacc")
        scr = big.tile([DI, LC], FP32, tag="scr")
        scr2 = big.tile([DI, LC], FP32, tag="scr2")
        qbuf = big.tile([DI, LC], FP32, tag="qbuf")
        dA_a = big.tile([DI, LC], FP32, tag="dA_a")
        dA_b = big.tile([DI, LC], FP32, tag="dA_b")
        rs_sb = big.tile([1, LC], FP32, tag="rs_sb")

        # ---- front 1x1 convs: proj = W @ feat + b  (own PSUM pool, closed
        # before the block loop so its 3 banks free up) ----
        with tc.tile_pool(name="frontps", bufs=1, space="PSUM") as frontps:
            for name, wname, bias_t, dst in (("rgb", "c1T", c1b, rgbp),
                                             ("dte", "c2T", c2b, dtep)):
                ps_f = frontps.tile([DM, LC], FP32, tag="front")
                for kt in range(C_IN // DI):
                    ktile = work.tile([DI, LC], FP32, tag="fr_in")
                    nc.sync.dma_start(ktile[:],
                                      ap(T[name])[kt * DI:(kt + 1) * DI, :])
                    wtile = work.tile([DI, DM], FP32, tag="fr_w")
                    nc.sync.dma_start(wtile[:],
                                      ap(T[wname])[kt * DI:(kt + 1) * DI, :])
                    for (c0, c1) in CHUNKS:
                        nc.tensor.matmul(ps_f[:, c0:c1], wtile[:],
                                         ktile[:, c0:c1],
                                         start=(kt == 0),
                                         stop=(kt == C_IN // DI - 1))
                nc.scalar.activation(dst[:], ps_f[:], F.Identity,
                                     bias=bias_t[:])

        mmps = stk.enter_context(
            tc.tile_pool(name="mmps", bufs=4, space="PSUM"))
        bcps = stk.enter_context(
            tc.tile_pool(name="bcps", bufs=4, space="PSUM"))

        # ---- resblocks ----
        for blk in range(NBLK):
            xin = rgbp if blk == 0 else xcur
            # RMSNorm: xn = xin * rsqrt(mean(xin^2)+eps) * rmsw
            nc.vector.tensor_tensor(scr[0:DM, :], xin[:], xin[:], AL.mult)
            for (c0, c1) in CHUNKS:
                w_ = c1 - c0
                ps_r = mmps.tile([DI, 512], FP32, tag="mm")
                nc.tensor.matmul(ps_r[0:1, :w_], ones64[:], scr[0:DM, c0:c1],
                                 start=True, stop=True)
                # rs = ln(sum/64 + eps) for now; exp(-.5 * ) after
                nc.scalar.activation(rs_sb[:, c0:c1], ps_r[0:1, :w_], F.Ln,
                                     scale=1.0 / DM, bias=epsb[:])
            nc.scalar.activation(rs_sb[:], rs_sb[:], F.Exp, scale=-0.5)
            for (c0, c1) in CHUNKS:
                w_ = c1 - c0
                ps_rb = mmps.tile([DI, 512], FP32, tag="mm")
                nc.tensor.matmul(ps_rb[0:DM, :w_], ones1x64[:],
                                 rs_sb[:, c0:c1], start=True, stop=True)
                nc.vector.tensor_tensor(xn[:, c0:c1], xin[:, c0:c1],
                                        ps_rb[0:DM, :w_], AL.mult)
            nc.vector.tensor_scalar_mul(xn[:], xn[:], rmsw[:])

            # in_proj: x1pre / z
            for (c0, c1) in CHUNKS:
                w_ = c1 - c0
                ps_x1 = mmps.tile([DI, 512], FP32, tag="mm")
                ps_z = mmps.tile([DI, 512], FP32, tag="mm")
                nc.tensor.matmul(ps_x1[:, :w_], ipT[:, 0:DI], xn[:, c0:c1],
                                 start=True, stop=True)
                nc.tensor.matmul(ps_z[:, :w_], ipT[:, DI:2 * DI], xn[:, c0:c1],
                                 start=True, stop=True)
                nc.vector.tensor_copy(x1p[:, c0:c1], ps_x1[:, :w_])
                nc.scalar.activation(zs[:, c0:c1], ps_z[:, :w_], F.Silu)

            # depthwise conv1d (window 3, zero pad) + bias, then silu
            nc.vector.tensor_scalar(scr[:], x1p[:], cw[:, 1:2], cb[:],
                                    AL.mult, AL.add)
            nc.vector.scalar_tensor_tensor(scr[:, 1:], x1p[:, :LC - 1],
                                           cw[:, 0:1], scr[:, 1:],
                                           AL.mult, AL.add)
            nc.vector.scalar_tensor_tensor(scr[:, :LC - 1], x1p[:, 1:],
                                           cw[:, 2:3], scr[:, :LC - 1],
                                           AL.mult, AL.add)
            nc.scalar.activation(x1c[:], scr[:], F.Silu)

            # x_proj -> delta_r (scr2 rows 0:32) and BC rows (scr rows 0:32)
            dr_sb = scr2
            bc_sb = scr
            for (c0, c1) in CHUNKS:
                w_ = c1 - c0
                ps_d = mmps.tile([DI, 512], FP32, tag="mm")
                ps_bc = mmps.tile([DI, 512], FP32, tag="mm")
                nc.tensor.matmul(ps_d[0:DTR, :w_], xpT[:, 0:DTR],
                                 x1c[:, c0:c1], start=True, stop=True)
                nc.tensor.matmul(ps_bc[0:2 * DS, :w_], xpT[:, DTR:2 * DTR],
                                 x1c[:, c0:c1], start=True, stop=True)
                nc.scalar.activation(dr_sb[0:DTR, c0:c1], ps_d[0:DTR, :w_],
                                     F.Copy)
                nc.vector.tensor_copy(bc_sb[0:2 * DS, c0:c1],
                                      ps_bc[0:2 * DS, :w_])

            # dt_proj + softplus: delta = ln(1 + exp(xp + dpb)); q = exp(-d)
            for (c0, c1) in CHUNKS:
                w_ = c1 - c0
                ps_dp = mmps.tile([DI, 512], FP32, tag="mm")
                nc.tensor.matmul(ps_dp[:, :w_], dpT[:], dr_sb[0:DTR, c0:c1],
                                 start=True, stop=True)
                nc.scalar.activation(delta[:, c0:c1], ps_dp[:, :w_], F.Exp,
                                     bias=dpb[:])
            nc.vector.tensor_scalar_add(delta[:], delta[:], 1.0)
            nc.scalar.activation(delta[:], delta[:], F.Ln)
            if power_chain and blk > 0:
                nc.scalar.activation(qbuf[:], delta[:], F.Exp, scale=-1.0)

            # dx = delta * x1c
            nc.vector.tensor_tensor(dxt[:], delta[:], x1c[:], AL.mult)

            # state update + y accumulation, per s
            prev_dA = None
            for s in range(DS):
                if blk > 0:
                    if power_chain:
                        if s == 0:
                            dA_s = qbuf
                        else:
                            dA_s = dA_a if (s % 2) else dA_b
                            nc.vector.tensor_tensor(dA_s[:], prev_dA[:],
                                                    qbuf[:], AL.mult)
                    else:
                        dA_s = dA_a if (s % 2) else dA_b
                        nc.scalar.activation(dA_s[:], delta[:], F.Exp,
                                             scale=Asb[:, s:s + 1])
                    prev_dA = dA_s
                for (c0, c1) in CHUNKS:
                    w_ = c1 - c0
                    ps_bmb = bcps.tile([DI, 512], FP32, tag="bc")
                    ps_cmb = bcps.tile([DI, 512], FP32, tag="bc")
                    nc.tensor.matmul(ps_bmb[:, :w_],
                                     sel[:, s * DI:(s + 1) * DI],
                                     bc_sb[0:2 * DS, c0:c1],
                                     start=True, stop=True)
                    nc.tensor.matmul(ps_cmb[:, :w_],
                                     sel[:, (DS + s) * DI:(DS + s + 1) * DI],
                                     bc_sb[0:2 * DS, c0:c1],
                                     start=True, stop=True)
                    hs = h_sb[:, s * LC + c0: s * LC + c1]
                    if blk == 0:
                        nc.vector.tensor_tensor(hs, dxt[:, c0:c1],
                                                ps_bmb[:, :w_], AL.mult)
                    else:
                        nc.vector.tensor_tensor(hs, hs, dA_s[:, c0:c1],
                                                AL.mult)
                        nc.vector.tensor_tensor(scr2[0:DI, c0:c1],
                                                dxt[:, c0:c1], ps_bmb[:, :w_],
                                                AL.mult)
                        nc.vector.tensor_tensor(hs, hs, scr2[0:DI, c0:c1],
                                                AL.add)
                    if s == 0:
                        nc.vector.tensor_tensor(yacc[:, c0:c1], hs,
                                                ps_cmb[:, :w_], AL.mult)
                    else:
                        nc.vector.tensor_tensor(scr2[0:DI, c0:c1], hs,
                                                ps_cmb[:, :w_], AL.mult)
                        nc.vector.tensor_tensor(yacc[:, c0:c1], yacc[:, c0:c1],
                                                scr2[0:DI, c0:c1], AL.add)
                if blk == NBLK - 1:
                    nc.sync.dma_start(ap(T["hout"])[:, s * LC:(s + 1) * LC],
                                      h_sb[:, s * LC:(s + 1) * LC])

            # y += D * x1c ; gate ; out_proj ; residual (+ next source)
            nc.vector.scalar_tensor_tensor(yacc[:], x1c[:], Dv[:], yacc[:],
                                           AL.mult, AL.add)
            nc.vector.tensor_tensor(yacc[:], yacc[:], zs[:], AL.mult)
            dst = o6 if blk == NBLK - 1 else xcur
            src_next = None if blk == NBLK - 1 else (dtep if blk % 2 == 0
                                                     else rgbp)
            for (c0, c1) in CHUNKS:
                w_ = c1 - c0
                ps_o = mmps.tile([DI, 512], FP32, tag="mm")
                nc.tensor.matmul(ps_o[0:DM, :w_], opT[:], yacc[:, c0:c1],
                                 start=True, stop=True)
                nc.vector.tensor_tensor(dst[:, c0:c1], ps_o[0:DM, :w_],
                                        xn[:, c0:c1], AL.add)
                if src_next is not None:
                    nc.vector.tensor_tensor(dst[:, c0:c1], dst[:, c0:c1],
                                            src_next[:, c0:c1], AL.add)

        # ---- final 1x1 conv (bias added on host) ----
        for mt in range(C_IN // DI):
            csb = work.tile([DI, LC], FP32, tag="c3out")
            for (c0, c1) in CHUNKS:
                w_ = c1 - c0
                ps_c = mmps.tile([DI, 512], FP32, tag="mm")
                nc.tensor.matmul(ps_c[:, :w_], c3T[:, mt * DI:(mt + 1) * DI],
                                 o6[:, c0:c1], start=True, stop=True)
                nc.vector.tensor_copy(csb[:, c0:c1], ps_c[:, :w_])
            nc.sync.dma_start(ap(T["outc"])[mt * DI:(mt + 1) * DI, :], csb[:])


def _build_program(power_chain, reps=1):
    _install_patches()
    nc = bass.Bass("TRN2", target_bir_lowering=False, debug=False)
    T = {}
    def din(name, shape):
        T[name] = nc.dram_tensor(name, shape, FP32, kind="ExternalInput")
    def dout(name, shape):
        T[name] = nc.dram_tensor(name, shape, FP32, kind="ExternalOutput")

    din("rgb", [C_IN, LC])
    din("dte", [C_IN, LC])
    din("c1T", [C_IN, DM])
    din("c2T", [C_IN, DM])
    din("c3T", [DM, C_IN])
    din("c1b", [DM, 1])
    din("c2b", [DM, 1])
    din("rmsw", [DM, 1])
    din("ipT", [DM, 2 * DI])
    din("cw", [DI, 3])
    din("cb", [DI, 1])
    din("xpT", [DI, 2 * DTR])
    din("dpT", [DTR, DI])
    din("dpb", [DI, 1])
    din("opT", [DI, DM])
    din("Dv", [DI, 1])
    din("sel", [2 * DS, 2 * DS * DI])
    if not power_chain:
        din("Asb", [DI, DS])
    dout("outc", [C_IN, LC])
    dout("hout", [DI, DS * LC])

    with tile.TileContext(nc) as tc:
        if reps > 1:
            with tc.For_i(0, reps, 1):
                _emit(nc, tc, T, power_chain)
        else:
            _emit(nc, tc, T, power_chain)
    return nc


def _get_program(power_chain, reps=1):
    key = (bool(power_chain), reps)
    if key not in _PROGRAM_CACHE:
        _PROGRAM_CACHE[key] = _build_program(power_chain, reps)
    return _PROGRAM_CACHE[key]


# ---------------------------------------------------------------------------
# Host wrapper
# ---------------------------------------------------------------------------

def _prep_in_maps(inputs):
    f32 = lambda a: np.ascontiguousarray(np.asarray(a, dtype=np.float32))
    rgb = f32(inputs["rgb_feat"]).reshape(B, C_IN, L)
    dte = f32(inputs["dte_feat"]).reshape(B, C_IN, L)

    A = -np.exp(np.asarray(inputs["A_log"], np.float64))  # [DI, DS]
    expect = -np.arange(1, DS + 1, dtype=np.float64)[None, :]
    power_chain = bool(np.allclose(A, np.broadcast_to(expect, A.shape),
                                   rtol=1e-5, atol=1e-5))

    # selector lhsT for B/C row broadcasts: one-hot [32, 128] per index
    sel = np.zeros((2 * DS, 2 * DS, DI), np.float32)
    for i in range(2 * DS):
        sel[i, i, :] = 1.0
    sel = sel.reshape(2 * DS, 2 * DS * DI)

    common = {
        "c1T": f32(inputs["conv1_w"]).T,
        "c2T": f32(inputs["conv2_w"]).T,
        "c3T": f32(inputs["conv3_w"]).T,
        "c1b": f32(inputs["conv1_b"]).reshape(DM, 1),
        "c2b": f32(inputs["conv2_b"]).reshape(DM, 1),
        "rmsw": f32(inputs["rms_w"]).reshape(DM, 1),
        "ipT": f32(inputs["in_proj_w"]).T,
        "cw": f32(inputs["conv1d_w"]).reshape(DI, 3),
        "cb": f32(inputs["conv1d_b"]).reshape(DI, 1),
        "xpT": f32(inputs["x_proj_w"]).T,
        "dpT": f32(inputs["dt_proj_w"]).T,
        "dpb": f32(inputs["dt_proj_b"]).reshape(DI, 1),
        "opT": f32(inputs["out_proj_w"]).T,
        "Dv": f32(inputs["D"]).reshape(DI, 1),
        "sel": sel,
    }
    if not power_chain:
        common["Asb"] = A.astype(np.float32)
    common = {k: np.ascontiguousarray(v) for k, v in common.items()}

    in_maps = []
    for core in range(N_CORES):
        b, hh = divmod(core, 2)
        lo = hh * LH - (HALO if hh else 0)
        m = dict(common)
        m["rgb"] = np.ascontiguousarray(rgb[b, :, lo:lo + LC])
        m["dte"] = np.ascontiguousarray(dte[b, :, lo:lo + LC])
        in_maps.append(m)
    return in_maps, power_chain


def _assemble(results, conv3_b):
    out = np.empty((B, C_IN, L), np.float32)
    h = np.empty((B, L, DI, DS), np.float32)
    for core in range(N_CORES):
        b, hh = divmod(core, 2)
        off = HALO if hh else 0
        oc = results[core]["outc"]                     # [C_IN, LC]
        out[b, :, hh * LH:(hh + 1) * LH] = oc[:, off:off + LH]
        hv = results[core]["hout"].reshape(DI, DS, LC)[:, :, off:off + LH]
        h[b, hh * LH:(hh + 1) * LH] = np.transpose(hv, (2, 0, 1))
    out += np.asarray(conv3_b, np.float32)[None, :, None]
    return out.reshape(B, C_IN, HWID, HWID), h


def kernel(**inputs):
    in_maps, power_chain = _prep_in_maps(inputs)
    nc = _get_program(power_chain)
    res = run_bass_kernel_spmd(nc, in_maps, core_ids=list(range(N_CORES)))
    return _assemble(res.results, inputs["conv3_b"])
